# revision 12
# baseline (speedup 1.0000x reference)
"""Trainium2 Bass kernel for nn_AlphaModel (gnn_message_passing).

Math (per edge n, P=3):
    M       = rel_mu[rels[n]] + rel_sigma[rels[n]] * eps_M[n]        [3,3]
    cp      = softmax(M @ child[n])                                  [3]
    masks   from row sums of child / prnt
    s       = 42 * max(.01, cos(prnt, cp)) / H(normalize(max(.01, prnt+cp)))
    alpha   = ((1-beta) * prnt + beta * cp) * s          (alpha_mask rows)
    c2c     = cp                                         (copy_mask rows)

Sharding: pure data parallel over the edge dim across 8 NeuronCores.
The rel_mu/rel_sigma tables in this problem are degenerate (all 20 rows
identical), so M = MU + sigma*eps needs no per-edge gather; we detect this
at runtime and bake the single 3x3 mu/sigma as immediates. General tables
fall back to a host-side gather of per-edge mu/sigma rows (extra DMA).

Layout: PLANAR (feature-major). Host transposes [N, F] -> [F, N] so every
on-chip operand is a contiguous [128, T] plane (2-D APs keep instruction
encoding room for sync waits; 3-D/4-D TT APs hit walrus "too many sync
wait commands").

Restructured identities used on device (validated vs reference to ~1e-7):
  - cos(p, cp) == cos(p, e) for e = exp(logits)  (scale invariance)
  - H(z/zs) = ln(zs) - (1/zs) * sum(z ln z)
  - norm guard: cos_raw = dot * rsqrt(sp*sc + 1e-30); dot==0 exactly when
    a norm is 0, so the guarded value matches the reference's where().
"""

import sys

sys.path.insert(0, "/opt/trn_rl_repo")

import numpy as np

import bass_rust
import concourse.bass as bass
import concourse.mybir as mybir
import concourse.tile as tile
from concourse.bass_utils import run_bass_kernel_spmd

# ---------------------------------------------------------------------------
# Post-pass: this container's walrus build rejects instructions carrying
# multiple sync waits ("Too many sync wait commands"). Raw-bass kernels work
# because standalone wait_ge lowers to a dedicated InstEventSemaphore; this
# rewrites a Tile-scheduled module into that form (waits hoisted onto
# event-sem instructions just before the owner, same engine queue).
# ---------------------------------------------------------------------------
_WSPLIT_N = [0]


def _wait_carrier(engine, wait):
    _WSPLIT_N[0] += 1
    ev = mybir.InstEventSemaphore(name=f"WSPLIT-{_WSPLIT_N[0]}", ins=[], outs=[])
    ev.engine = engine
    ev.sync_info = bass_rust.SyncInfo(on_wait=[wait], on_update=[])
    return ev


def split_waits(nc, keep_on_control=1):
    for fn in nc.m.functions:
        for blk in fn.blocks:
            out = []
            for ins in blk.instructions:
                si = ins.sync_info
                waits = list(si.on_wait) if (si and si.on_wait) else []
                is_ctrl = type(ins).__name__ in (
                    "InstEventSemaphore",
                    "InstDrain",
                    "InstUnconditionalBranch",
                    "InstCompareAndBranch",
                    "InstBranchHint",
                )
                keep = keep_on_control if is_ctrl else 0
                if len(waits) > keep:
                    cut = len(waits) - keep
                    for w in waits[:cut]:
                        out.append(_wait_carrier(ins.engine, w))
                    ins.sync_info = bass_rust.SyncInfo(
                        on_wait=waits[cut:], on_update=list(si.on_update or [])
                    )
                out.append(ins)
            blk.instructions = out
    return nc

PARTS = 128
T_COL = 391          # edges per partition per tile
N_TILES = 10         # tiles per core
N_CORES = 8

f32 = mybir.dt.float32
Alu = mybir.AluOpType
Act = mybir.ActivationFunctionType

LAST_RESULT = None  # BassKernelResults of the most recent run (for test.py)

# setup_inputs() defaults, used if the harness omits the tiny tables.
_MU_DEFAULT = np.eye(3, dtype=np.float32)
_MU_DEFAULT[1, :] = [-0.25, 0.5, -0.25]


def build_graph(mu9, sg9, general, t_col=T_COL, n_tiles=N_TILES):
    """Build the per-core Bass graph (planar layout).

    mu9/sg9: 9-element mu/sigma rows (path A, degenerate tables -> immediates).
    general: per-edge murow/sgrow [9, NS] arrive as extra planar inputs.
    """
    T = t_col
    NS = PARTS * T * n_tiles
    sigma_is_one = (not general) and bool(np.all(sg9 == 1.0))

    nc = bass.Bass()
    prnt_h = nc.declare_dram_parameter("prnt", [3, NS], f32, isOutput=False)
    child_h = nc.declare_dram_parameter("child", [3, NS], f32, isOutput=False)
    eps_h = nc.declare_dram_parameter("eps", [9, NS], f32, isOutput=False)
    beta_h = nc.declare_dram_parameter("beta", [1, NS], f32, isOutput=False)
    if general:
        murow_h = nc.declare_dram_parameter("murow", [9, NS], f32, isOutput=False)
        sgrow_h = nc.declare_dram_parameter("sgrow", [9, NS], f32, isOutput=False)
    c2c_h = nc.declare_dram_parameter("c2c", [3, NS], f32, isOutput=True)
    alpha_h = nc.declare_dram_parameter("alpha", [3, NS], f32, isOutput=True)
    m2_h = nc.declare_dram_parameter("m2", [2, NS], f32, isOutput=True)

    def dram_tile(handle, nplanes, base):
        # [F, NS] DRAM rows -> [128, F, T] AP (partition p, planes along free)
        return (
            handle[:, base : base + PARTS * T]
            .rearrange("k (p t) -> k p t", p=PARTS)
            .transpose([1, 0, 2])
        )

    with tile.TileContext(nc) as tc:
        with (
            tc.tile_pool(name="const", bufs=1) as cpool,
            tc.tile_pool(name="io", bufs=2) as io,
            tc.tile_pool(name="work", bufs=1) as wk,
        ):
            # [128,1] bias tiles for scalar-engine activations
            bias_m01 = cpool.tile([PARTS, 1], f32)
            nc.gpsimd.memset(bias_m01[:], -0.01)
            bias_p01 = cpool.tile([PARTS, 1], f32)
            nc.gpsimd.memset(bias_p01[:], 0.01)
            bias_tiny = cpool.tile([PARTS, 1], f32)
            nc.gpsimd.memset(bias_tiny[:], 1e-30)
            bias_ln42 = cpool.tile([PARTS, 1], f32)
            nc.gpsimd.memset(bias_ln42[:], float(np.log(42.0)))

            for it in range(n_tiles):
                base = it * PARTS * T

                # ---- loads ----
                c_t = io.tile([PARTS, 3 * T], f32, tag="c_t")
                p_t = io.tile([PARTS, 3 * T], f32, tag="p_t")
                e_t = io.tile([PARTS, 9 * T], f32, tag="e_t")
                b_t = io.tile([PARTS, T], f32, tag="b_t")
                nc.gpsimd.dma_start(c_t[:], dram_tile(child_h, 3, base))
                nc.gpsimd.dma_start(p_t[:], dram_tile(prnt_h, 3, base))
                nc.gpsimd.dma_start(e_t[:], dram_tile(eps_h, 9, base))
                nc.gpsimd.dma_start(b_t[:], dram_tile(beta_h, 1, base))
                if general:
                    mr_t = io.tile([PARTS, 9 * T], f32, tag="mr_t")
                    sr_t = io.tile([PARTS, 9 * T], f32, tag="sr_t")
                    nc.gpsimd.dma_start(mr_t[:], dram_tile(murow_h, 9, base))
                    nc.gpsimd.dma_start(sr_t[:], dram_tile(sgrow_h, 9, base))

                oc2c = io.tile([PARTS, 3 * T], f32, tag="oc2c")
                oalp = io.tile([PARTS, 3 * T], f32, tag="oalp")
                om2 = io.tile([PARTS, 2 * T], f32, tag="om2")

                def pl(tl, k):
                    return tl[:, k * T : (k + 1) * T]

                cpl = [pl(c_t, j) for j in range(3)]
                ppl = [pl(p_t, j) for j in range(3)]
                epl = [pl(e_t, k) for k in range(9)]

                # ---- logits_i = sum_j (mu_ij + sg_ij*eps_ij) * c_j ----
                if general:
                    # e_t = e_t * sgrow + murow  (in place, whole tile)
                    nc.gpsimd.tensor_tensor(e_t[:], e_t[:], sr_t[:], Alu.mult)
                    nc.gpsimd.tensor_tensor(e_t[:], e_t[:], mr_t[:], Alu.add)
                elif not sigma_is_one:
                    for k in range(9):
                        nc.gpsimd.tensor_scalar(
                            epl[k], epl[k], float(sg9[k]), float(mu9[k]),
                            Alu.mult, Alu.add,
                        )

                lg = wk.tile([PARTS, 3 * T], f32, tag="lg")
                tmp = wk.tile([PARTS, T], f32, tag="tmp")
                for i in range(3):
                    lgi = pl(lg, i)
                    if general or not sigma_is_one:
                        # M already materialized in e_t
                        nc.vector.tensor_tensor(lgi, epl[3 * i], cpl[0], Alu.mult)
                        nc.vector.tensor_tensor(tmp[:], epl[3 * i + 1], cpl[1], Alu.mult)
                        nc.vector.tensor_tensor(lgi, lgi, tmp[:], Alu.add)
                        nc.vector.tensor_tensor(tmp[:], epl[3 * i + 2], cpl[2], Alu.mult)
                        nc.vector.tensor_tensor(lgi, lgi, tmp[:], Alu.add)
                    else:
                        # lg_i = ((eps_i0+mu_i0)*c0 + (eps_i1+mu_i1)*c1) + ...
                        nc.vector.scalar_tensor_tensor(
                            lgi, epl[3 * i], float(mu9[3 * i]), cpl[0],
                            Alu.add, Alu.mult,
                        )
                        nc.vector.scalar_tensor_tensor(
                            tmp[:], epl[3 * i + 1], float(mu9[3 * i + 1]), cpl[1],
                            Alu.add, Alu.mult,
                        )
                        nc.vector.tensor_tensor(lgi, lgi, tmp[:], Alu.add)
                        nc.vector.scalar_tensor_tensor(
                            tmp[:], epl[3 * i + 2], float(mu9[3 * i + 2]), cpl[2],
                            Alu.add, Alu.mult,
                        )
                        nc.vector.tensor_tensor(lgi, lgi, tmp[:], Alu.add)

                # ---- softmax numerator & cp ----
                ex = lg  # exp in place
                nc.scalar.activation(ex[:], lg[:], Act.Exp)
                expl = [pl(ex, i) for i in range(3)]
                se = wk.tile([PARTS, T], f32, tag="se")
                nc.vector.tensor_tensor(se[:], expl[0], expl[1], Alu.add)
                nc.vector.tensor_tensor(se[:], se[:], expl[2], Alu.add)
                r = wk.tile([PARTS, T], f32, tag="r")
                nc.scalar.activation(r[:], se[:], Act.Ln)
                nc.scalar.activation(r[:], r[:], Act.Exp, scale=-1.0)
                cp = wk.tile([PARTS, 3 * T], f32, tag="cp")
                cppl = [pl(cp, i) for i in range(3)]
                for i in range(3):
                    nc.vector.tensor_tensor(cppl[i], expl[i], r[:], Alu.mult)

                # ---- masks (exact equality semantics as reference) ----
                csum = wk.tile([PARTS, T], f32, tag="csum")
                nc.vector.tensor_tensor(csum[:], cpl[0], cpl[1], Alu.add)
                nc.vector.tensor_tensor(csum[:], csum[:], cpl[2], Alu.add)
                psum = wk.tile([PARTS, T], f32, tag="psum")
                nc.vector.tensor_tensor(psum[:], ppl[0], ppl[1], Alu.add)
                nc.vector.tensor_tensor(psum[:], psum[:], ppl[2], Alu.add)
                cm = wk.tile([PARTS, T], f32, tag="cm")
                nc.vector.tensor_scalar(cm[:], csum[:], 0.0, None, Alu.not_equal)
                copym = pl(om2, 0)
                amask = pl(om2, 1)
                nc.vector.scalar_tensor_tensor(
                    copym, psum[:], 0.0, cm[:], Alu.is_equal, Alu.mult
                )
                nc.vector.scalar_tensor_tensor(
                    amask, psum[:], 0.0, cm[:], Alu.not_equal, Alu.mult
                )

                # ---- entropy branch: z = max(.01, p+cp), zp = z - .01 ----
                zp = wk.tile([PARTS, 3 * T], f32, tag="zp")
                zppl = [pl(zp, i) for i in range(3)]
                for i in range(3):
                    nc.gpsimd.tensor_tensor(zppl[i], ppl[i], cppl[i], Alu.add)
                nc.scalar.activation(zp[:], zp[:], Act.Relu, bias=bias_m01[:])
                zs = wk.tile([PARTS, T], f32, tag="zs")
                nc.vector.tensor_tensor(zs[:], zppl[0], zppl[1], Alu.add)
                nc.vector.scalar_tensor_tensor(
                    zs[:], zppl[2], 0.03, zs[:], Alu.add, Alu.add
                )
                zr = wk.tile([PARTS, T], f32, tag="zr")
                nc.scalar.activation(zr[:], zs[:], Act.Ln)
                nc.scalar.activation(zr[:], zr[:], Act.Exp, scale=-1.0)
                lnz = wk.tile([PARTS, 3 * T], f32, tag="lnz")
                nc.scalar.activation(lnz[:], zp[:], Act.Ln, bias=bias_p01[:])
                lnzs = wk.tile([PARTS, T], f32, tag="lnzs")
                nc.scalar.activation(lnzs[:], zs[:], Act.Ln)
                # zlnz = (zp + .01) * lnz  (in place over lnz)
                nc.vector.scalar_tensor_tensor(
                    lnz[:], zp[:], 0.01, lnz[:], Alu.add, Alu.mult
                )
                zlpl = [pl(lnz, i) for i in range(3)]
                tsum = wk.tile([PARTS, T], f32, tag="tsum")
                nc.vector.tensor_tensor(tsum[:], zlpl[0], zlpl[1], Alu.add)
                nc.vector.tensor_tensor(tsum[:], tsum[:], zlpl[2], Alu.add)
                ent = wk.tile([PARTS, T], f32, tag="ent")
                nc.vector.tensor_tensor(ent[:], zr[:], tsum[:], Alu.mult)
                nc.vector.scalar_tensor_tensor(
                    ent[:], ent[:], -1.0, lnzs[:], Alu.mult, Alu.add
                )
                esr = wk.tile([PARTS, T], f32, tag="esr")  # holds 42/ent
                nc.scalar.activation(esr[:], ent[:], Act.Ln)
                nc.scalar.activation(
                    esr[:], esr[:], Act.Exp, scale=-1.0, bias=bias_ln42[:]
                )

                # ---- cosine branch (on unnormalized ex; scale invariant) ----
                dp = wk.tile([PARTS, T], f32, tag="dp")
                nc.vector.tensor_tensor(dp[:], ppl[0], expl[0], Alu.mult)
                nc.vector.tensor_tensor(tmp[:], ppl[1], expl[1], Alu.mult)
                nc.vector.tensor_tensor(dp[:], dp[:], tmp[:], Alu.add)
                nc.vector.tensor_tensor(tmp[:], ppl[2], expl[2], Alu.mult)
                nc.vector.tensor_tensor(dp[:], dp[:], tmp[:], Alu.add)

                sq = wk.tile([PARTS, 3 * T], f32, tag="sq")
                nc.scalar.activation(sq[:], p_t[:], Act.Square)
                sqpl = [pl(sq, i) for i in range(3)]
                sp = wk.tile([PARTS, T], f32, tag="sp")
                nc.vector.tensor_tensor(sp[:], sqpl[0], sqpl[1], Alu.add)
                nc.vector.tensor_tensor(sp[:], sp[:], sqpl[2], Alu.add)
                nc.scalar.activation(sq[:], ex[:], Act.Square)
                sc = wk.tile([PARTS, T], f32, tag="sc")
                nc.vector.tensor_tensor(sc[:], sqpl[0], sqpl[1], Alu.add)
                nc.vector.tensor_tensor(sc[:], sc[:], sqpl[2], Alu.add)

                mn = wk.tile([PARTS, T], f32, tag="mn")
                nc.vector.tensor_tensor(mn[:], sp[:], sc[:], Alu.mult)
                nc.scalar.activation(mn[:], mn[:], Act.Ln, bias=bias_tiny[:])
                drr = wk.tile([PARTS, T], f32, tag="drr")
                nc.scalar.activation(drr[:], mn[:], Act.Exp, scale=-0.5)

                # sm = (max(.01, dp*drr) * esr * 42) * amask
                sm = wk.tile([PARTS, T], f32, tag="sm")
                nc.vector.tensor_tensor(sm[:], dp[:], drr[:], Alu.mult)
                nc.vector.scalar_tensor_tensor(
                    sm[:], sm[:], 0.01, esr[:], Alu.max, Alu.mult
                )
                nc.vector.tensor_tensor(sm[:], sm[:], amask, Alu.mult)

                # ---- alpha = (p + beta*(cp - p)) * sm ; c2c = cp * copym ----
                for i in range(3):
                    oa = pl(oalp, i)
                    nc.vector.tensor_tensor(oa, cppl[i], ppl[i], Alu.subtract)
                    nc.gpsimd.tensor_tensor(oa, oa, b_t[:], Alu.mult)
                    nc.gpsimd.tensor_tensor(oa, oa, ppl[i], Alu.add)
                    nc.vector.tensor_tensor(oa, oa, sm[:], Alu.mult)
                    nc.vector.tensor_tensor(
                        pl(oc2c, i), cppl[i], copym, Alu.mult
                    )

                # ---- stores ----
                nc.gpsimd.dma_start(dram_tile(c2c_h, 3, base), oc2c[:])
                nc.gpsimd.dma_start(dram_tile(alpha_h, 3, base), oalp[:])
                nc.gpsimd.dma_start(dram_tile(m2_h, 2, base), om2[:])

    split_waits(nc)
    return nc, NS


def _shard_planar(arr, ns):
    """[N, F] -> list of 8 contiguous [F, ns] planar shards (zero-padded)."""
    n = arr.shape[0]
    f = arr.shape[1]
    shards = []
    for i in range(N_CORES):
        lo, hi = i * ns, (i + 1) * ns
        out = np.zeros((f, ns), dtype=np.float32)
        m = max(0, min(hi, n) - lo)
        if m > 0:
            out[:, :m] = arr[lo : lo + m].T
        shards.append(out)
    return shards


def kernel(
    prnt_probs,
    child_probs,
    eps_M,
    beta,
    rels=None,
    rel_mu=None,
    rel_sigma=None,
    **_unused,
):
    prnt = np.asarray(prnt_probs, dtype=np.float32)
    child = np.asarray(child_probs, dtype=np.float32)
    n = prnt.shape[0]
    eps = np.asarray(eps_M, dtype=np.float32).reshape(n, 9)
    bet = np.asarray(beta, dtype=np.float32).reshape(n, 1)

    if rel_mu is None:
        rel_mu = np.tile(_MU_DEFAULT, (20, 1)).reshape(20, 3, 3)
    if rel_sigma is None:
        rel_sigma = np.ones((20, 3, 3), dtype=np.float32)
    rel_mu = np.asarray(rel_mu, dtype=np.float32)
    rel_sigma = np.asarray(rel_sigma, dtype=np.float32)

    degenerate = bool(
        np.all(rel_mu == rel_mu[0:1]) and np.all(rel_sigma == rel_sigma[0:1])
    )

    n_tiles = max(1, -(-n // (N_CORES * PARTS * T_COL)))  # ceil
    nc, ns = build_graph(
        rel_mu[0].reshape(9),
        rel_sigma[0].reshape(9),
        general=not degenerate,
        n_tiles=n_tiles,
    )

    p_sh = _shard_planar(prnt, ns)
    c_sh = _shard_planar(child, ns)
    e_sh = _shard_planar(eps, ns)
    b_sh = _shard_planar(bet, ns)
    if not degenerate:
        ridx = np.asarray(rels).astype(np.int64)
        mr_sh = _shard_planar(rel_mu.reshape(20, 9)[ridx], ns)
        sr_sh = _shard_planar(rel_sigma.reshape(20, 9)[ridx], ns)
    in_maps = []
    for i in range(N_CORES):
        m = {"prnt": p_sh[i], "child": c_sh[i], "eps": e_sh[i], "beta": b_sh[i]}
        if not degenerate:
            m["murow"] = mr_sh[i]
            m["sgrow"] = sr_sh[i]
        in_maps.append(m)

    import os

    trace = bool(os.environ.get("ALPHA_KERNEL_TRACE"))
    res = run_bass_kernel_spmd(
        nc, in_maps, core_ids=list(range(N_CORES)), trace=trace
    )
    global LAST_RESULT
    LAST_RESULT = res
    outs = res.results

    c2c = np.concatenate([outs[i]["c2c"] for i in range(N_CORES)], axis=1)
    alpha = np.concatenate([outs[i]["alpha"] for i in range(N_CORES)], axis=1)
    m2 = np.concatenate([outs[i]["m2"] for i in range(N_CORES)], axis=1)
    c2c = np.ascontiguousarray(c2c.T[:n])
    alpha = np.ascontiguousarray(alpha.T[:n])
    copy_mask = m2[0, :n] != 0
    alpha_mask = m2[1, :n] != 0
    return copy_mask, c2c, alpha_mask, alpha
